# revision 36
# baseline (speedup 1.0000x reference)
"""Trainium2 Bass kernel for nn_ConvFCLIFNet.

Pipeline: x_seq (T=64, B=512, 1, 28, 28) -> conv2x2(valid) -> FC(729) -> LIF
scan over T -> spike sequence (T, B, 729) in {0.0, 1.0}.

Strategy (v4)
-------------
- The 2x2 conv is applied HOST-side (f32, exact): the device matmul then
  contracts over 729 conv pixels + 1 bias row = 730 rows -> 6 k-tiles of
  (5x128 + 90) instead of 785 -> 7. That cuts tensor-engine time and x
  traffic by ~14% each.
- fp16 x and weights: the PE truncates f32 ("f32r") operands to e10m11
  internally anyway; fp16 is the same 11-bit mantissa (RNE, unbiased vs
  f32r's truncation) at HALF the HBM bytes. Verified by numpy simulation:
  ~1.7k spike flips vs a ~2.9k budget at the 2e-2 rel-err gate.
- y*0.5 = x_aug @ W_aug with the 1/tau fold and bias row (row 729) baked
  into W_aug host-side, as in v2.
- Data-parallel over 8 NeuronCores: 64 samples each.
- T=64 in quanta sized so the PE is fed exactly as fast as the startup
  DMA pipe (~340 GB/s) can deliver x, and the tail tapers to a 2-step
  final quantum (short serial LIF+DMA epilogue). Within a quantum the
  matmul loop is 512-col-chunk-outer (8 timesteps), j middle, k-tile
  inner into single-bank PSUM tiles; Scalar drains each (chunk, j) to
  an SBUF y-buffer immediately, so the LIF scan can start on a chunk's
  timesteps while the PE is still on the next chunk.
- LIF scan: ONE custom DVE op per timestep (sentinel-encoded state q,
  q = (w >= 1) ? SENT : 0.5*w) over a 32-slot state ring. The raw f32
  ring is DMA'd out in small batches ISSUED ON THE SYNC QUEUE with a
  2-quantum emission lag early (LIF semaphores pre-cleared -> no
  head-of-line blocking of x prefetches) collapsing to a 1-quantum lag
  in the tail (with x prefetched 2 quanta ahead there), so the final
  timesteps' output pipelines behind the LIF chain instead of draining
  serially after it. GpSimd stays fully idle, which removes its
  expensive DGE teardown drain. Host decodes spike = (q == SENT).
- ~34 warm-up matmuls on a zeroed scratch tile run while the first x
  quantum loads, so the PE's HAM clock-gate reaches 2.4 GHz before real
  work starts.
"""
import numpy as np

import concourse.bacc as bacc
import concourse.mybir as mybir
import concourse.tile as tile
from concourse.bass_utils import run_bass_kernel_spmd

# ---------------------------------------------------------------- constants
T, B, H, W = 64, 512, 28, 28
NF = 729                # fc features (= conv pixels 27*27)
NROWS = NF + 1          # 730 contraction rows (conv pixels + bias)
NCORES = 8
BS = B // NCORES        # 64 samples per core
NJ = 6                  # feature chunks of 128 (768 padded)
KT = 6                  # contraction k-tiles: 5 x 128 + 90
KROW = 128              # rows per k-tile (kt < 5)
KTAIL = NROWS - 5 * KROW   # 90 (89 pixels + bias row)
QUANTA = (8, 8, 12, 16, 12, 4, 2, 2)  # timesteps per quantum: small leading
                                   # quanta keep the PE fed while the DMA
                                   # pipe ramps; small tail quanta shrink
                                   # the serial LIF+DMA epilogue
OUTB = 4                # timesteps per output DMA
# which quanta's output DMAs are emitted during which loop iteration
# (lag-2 early, lag-1 for the tail; the final quantum is emitted post-loop)
EMIT_AT = {2: (0,), 3: (1,), 4: (2,), 5: (3, 4), 6: (5,), 7: (6,)}
NWARM = 24              # PE warm-up matmuls
SENT = float(2 ** 20)

_CACHE = {}


def _chunks(ns, step=512):
    """(offset, width) pairs covering ns columns in <=step slices."""
    out = []
    o = 0
    while o < ns:
        out.append((o, min(step, ns - o)))
        o += step
    return out

# ------------------------------------------------------------ custom DVE op

def _register_lif_op():
    from concourse.dve_spec import Spec, Src0, Src1, C0, C1, Zero, One, select, eq, lower
    from concourse.dve_uop import DveOpSpec
    from concourse import dve_ops

    name = "LIF_STEP_ANT"
    for op in dve_ops.OPS:
        if op.name == name:
            return op

    def _ref(in0, in1, s0, s1, imm2=None):
        u = np.where(in1 == s0, 0.0, in1).astype(np.float32)
        w = (in0 + u).astype(np.float32)
        return np.where(w >= 1.0, np.float32(s0), (w * np.float32(s1)).astype(np.float32))

    _u = select(eq(Src1, C0), Zero, Src1)
    _w = Src0 + _u
    spec = Spec(body=select(_w >= One, C0, _w * C1), reference=_ref)

    row = dve_ops._CUSTOM_DVE_ROW_BASE + len(dve_ops.OPS)
    assert row < 0x20
    dve_ops._SUB_OPCODE_FOR_NAME[name] = row
    shas = {}
    for ver in ("v3", "v4"):
        s = DveOpSpec(name=name, opcode=row, uops=lower(spec, ver=ver), rd1_en=True)
        shas[ver] = s.sha(ver)
    op = dve_ops.DveOp(name, spec, subdim=False, uops_sha=shas)
    dve_ops.OPS.append(op)
    dve_ops.CUSTOM_DVE_SPECS[name] = spec
    return op

# ------------------------------------------------------------- device build

def _build():
    lif = _register_lif_op()
    nc = bacc.Bacc(None, target_bir_lowering=False, debug=False)
    f32, f16, f8 = mybir.dt.float32, mybir.dt.float16, mybir.dt.float8e5
    NTOT = T * BS  # 4096 moving columns total
    with tile.TileContext(nc) as tc:
        with tc.tile_pool(name="dram", bufs=1, space="DRAM") as dram, \
             tc.tile_pool(name="consts", bufs=1) as consts, \
             tc.tile_pool(name="xpool", bufs=3) as xpool, \
             tc.tile_pool(name="ypool", bufs=3) as ypool, \
             tc.tile_pool(name="pspool", bufs=8, space="PSUM") as pspool:
            # x pre-tiled host-side to [partition, kt, col] so a quantum load
            # is ONE 3D access pattern; full-128-partition DMAs only
            x_in = dram.tile([128, KT, NTOT], f16, kind="ExternalInput",
                             name="x_in", uniquify=False)
            w_in = dram.tile([NJ, 128, KT, 128], f16, kind="ExternalInput",
                             name="w_in", uniquify=False)
            out = dram.tile([128, T, NJ, BS], f32, kind="ExternalOutput",
                            name="out", uniquify=False)

            wsb = consts.tile([128, NJ, KT, 128], f16)

            # 32-slot ring of LIF state; slot t%32 holds q after step t.
            # Slot 31 doubles as the zero initial state. The raw sentinel-
            # encoded f32 state is DMA'd out directly (no on-device spike
            # decode); the host decodes spike = (q == SENT). 32 slots (2
            # quanta of WAR slack) let the output DMAs be EMITTED two
            # quanta late on the Sync queue: their LIF semaphores are
            # already cleared when they reach the queue head, so they
            # never head-of-line-block the x prefetches, and GpSimd stays
            # fully idle (no expensive DGE teardown drain).
            NRING = 32
            qring = consts.tile([128, NRING, NJ * BS], f32)
            nc.vector.memset(qring[:, NRING - 1, :], 0.0)

            def emit_outs(c, ob=OUTB):
                """Output DMAs for quantum c's timesteps, ob per descriptor."""
                for tb in range(t0s[c], t0s[c + 1], ob):
                    nc.sync.dma_start(
                        out=out[:, tb:tb + ob, :, :],
                        in_=qring[:, tb % NRING:tb % NRING + ob, :])

            # PE warm-up scratch (zeros; results discarded)
            warm = consts.tile([128, 128], f16)
            nc.vector.memset(warm[:, :], 0.0)

            t0s = np.cumsum((0,) + QUANTA)

            def load_x(c, kt_split=False):
                """x DMA for quantum c (optionally split by k-tile so the
                first matmuls can start on partial data)."""
                tq = QUANTA[c]
                c0 = t0s[c] * BS
                x_sb = xpool.tile([128, KT, 16 * BS], f16, name="x_sb", tag="x")
                splits = ((0, 2), (2, 4), (4, KT)) if kt_split else ((0, KT),)
                for k0, k1 in splits:
                    nc.sync.dma_start(
                        out=x_sb[:, k0:k1, 0:tq * BS],
                        in_=x_in[:, k0:k1, c0:c0 + tq * BS],
                    )
                return x_sb

            # startup order: w0 first (smallest blocker for the j=0 chain),
            # then x0 split by k-tile, then the remaining weights; warm-up
            # matmuls keep the PE HAM busy while those DMAs land.
            wps = pspool.tile([128, 512], f32, name="ps", tag="ps")
            for i in range(NWARM):
                nc.tensor.matmul(wps[:, 0:128], lhsT=warm[0:128, 0:128],
                                 rhs=warm[:, 0:128], start=True, stop=True)
            # w0 and x0 in fine k-tile pieces: the first matmul only
            # needs (j0, kt0-1) weights + the kt0-1 x rows, so it starts
            # ~1.5us earlier than waiting for the full 0.59 MB
            nc.sync.dma_start(out=wsb[:, 0, 0:2, :], in_=w_in[0][:, 0:2, :])
            x0 = load_x(0, kt_split=True)
            nc.sync.dma_start(out=wsb[:, 0, 2:KT, :], in_=w_in[0][:, 2:KT, :])
            for j in range(1, NJ):
                nc.sync.dma_start(out=wsb[:, j, :, :], in_=w_in[j])
            x_tiles = {0: x0}

            for c, tq in enumerate(QUANTA):
                ns = tq * BS                      # moving cols this quantum
                x_sb = x_tiles.pop(c)
                # prefetch one quantum ahead early on, two ahead in the
                # tail so the small late x loads are never queued behind
                # lag-1 output emissions on the Sync queue
                for ahead in (1, 2):
                    cn = c + ahead
                    if (cn < len(QUANTA) and cn not in x_tiles
                            and (ahead == 1 or c >= 3)):
                        x_tiles[cn] = load_x(cn)
                # Output emission schedule: lag-2 for the big early quanta
                # (their long LIF chains need the slack so the DMAs never
                # wait at the Sync queue head), collapsing to lag-1 for the
                # small tail quanta (their LIF semaphores clear quickly) so
                # the 3.9 MB of tail output PIPELINES behind the LIF chain
                # instead of draining serially after it. Small ob batches in
                # the tail stream each pair of steps as soon as LIF lands.
                for cq in EMIT_AT.get(c, ()):
                    emit_outs(cq, ob=4 if QUANTA[cq] > 4 else 2)
                y_sb = ypool.tile([128, NJ, 16 * BS], f32, name="y_sb", tag="y")
                # chunk-outer: each 512-col chunk (8 timesteps) finishes all
                # 6 j accumulations + drains before the next chunk, so the
                # LIF scan starts half a quantum earlier
                for o, wd in _chunks(ns):
                    for j in range(NJ):
                        ps = pspool.tile([128, 512], f32, name="ps", tag="ps")
                        for kt in range(KT):
                            rows = KROW if kt < 5 else KTAIL
                            nc.tensor.matmul(
                                ps[:, 0:wd],
                                lhsT=wsb[0:rows, j, kt, :],
                                rhs=x_sb[0:rows, kt, o:o + wd],
                                start=(kt == 0), stop=(kt == KT - 1),
                            )
                        # drain PSUM slot to SBUF right away (GpSimd can't
                        # read PSUM on TRN2, so Scalar does all drains)
                        nc.scalar.copy(out=y_sb[:, j, o:o + wd],
                                       in_=ps[:, 0:wd])

                # LIF scan over this quantum's timesteps (from SBUF)
                for tl in range(tq):
                    t = t0s[c] + tl
                    nc.vector._custom_dve(
                        lif,
                        out=qring[:, t % NRING, :],
                        in0=y_sb[:, :, tl * BS:(tl + 1) * BS],
                        in1=qring[:, (t - 1) % NRING, :],
                        s0=SENT, s1=0.5,
                    )

            # tail: the final quantum's outputs, small batches so the
            # last LIF -> DMA chain is short
            emit_outs(len(QUANTA) - 1, ob=2)
    nc.compile()
    return nc

# --------------------------------------------------------------- host side

def _prep_weights(fc_w, fc_b):
    """w_in [NJ, 128, KT, 128] fp16: contraction rows (730 = 5*128 + 90) in
    partition-major per-j blocks; cols = 768 features (729 + pad); scaled by
    0.5 (tau fold). Bias lives at global row 729 (kt=5, p=89)."""
    w_aug = np.zeros((KT * 128, NJ * 128), np.float32)
    w_aug[:NF, :NF] = 0.5 * fc_w.astype(np.float32).T   # [in_pixel, out_feat]
    w_aug[NF, :NF] = 0.5 * fc_b.astype(np.float32)
    w4 = w_aug.reshape(KT, 128, NJ, 128).astype(np.float16)
    return np.ascontiguousarray(w4.transpose(2, 1, 0, 3))  # [NJ, 128, KT, 128]

def _prep_x(x_seq, conv_w):
    """Host conv (f32, exact) + per-core pre-tiled inputs
    [NCORES][128, KT, T*BS] fp16, cols t-major.

    Partition p of k-tile kt holds contraction row kt*128 + p (rows 0..729 =
    729 conv pixels + bias row of ones); unused pad partitions are zero."""
    cw = conv_w.reshape(2, 2).astype(np.float32)
    x = np.asarray(x_seq, dtype=np.float32).reshape(T, B, H, W)
    xc = (cw[0, 0] * x[:, :, :27, :27] + cw[0, 1] * x[:, :, :27, 1:] +
          cw[1, 0] * x[:, :, 1:, :27] + cw[1, 1] * x[:, :, 1:, 1:])
    xc = xc.reshape(T, NCORES, BS, NF).astype(np.float16)
    xt = xc.transpose(1, 3, 0, 2).reshape(NCORES, NF, T * BS)
    xp = np.zeros((NCORES, 128, KT, T * BS), np.float16)
    for kt in range(KT):
        g0 = kt * KROW
        npx = min(KROW, NF - g0)                      # pixel rows in this kt
        xp[:, 0:npx, kt, :] = xt[:, g0:g0 + npx, :]
    xp[:, NF - 5 * KROW, 5, :] = 1.0                  # bias row (global 729)
    return xp

def kernel(x_seq, conv_w, fc_w, fc_b):
    # coerce to numpy up front (the caller may pass jax arrays; all host
    # prep must stay on the CPU)
    x_seq = np.asarray(x_seq, dtype=np.float32)
    conv_w = np.asarray(conv_w, dtype=np.float32)
    fc_w = np.asarray(fc_w, dtype=np.float32)
    fc_b = np.asarray(fc_b, dtype=np.float32)
    if "nc" not in _CACHE:
        _CACHE["nc"] = _build()
    nc = _CACHE["nc"]
    w_in = _prep_weights(fc_w, fc_b)
    xp = _prep_x(x_seq, conv_w)
    in_maps = [{"x_in": np.ascontiguousarray(xp[c]), "w_in": w_in}
               for c in range(NCORES)]
    res = run_bass_kernel_spmd(nc, in_maps, core_ids=list(range(NCORES)))
    _CACHE["last_res"] = res
    full = np.empty((T, B, NF), np.float32)
    for c in range(NCORES):
        o = res.results[c]["out"]                     # [128, T, NJ, BS] f32
        # spike decode: q == SENT exactly iff the neuron fired this step
        s = (o == np.float32(SENT)).astype(np.float32)
        # feature f = j*128 + p ; sample b
        full[:, c * BS:(c + 1) * BS, :] = (
            s.transpose(1, 3, 2, 0).reshape(T, BS, NJ * 128)[:, :, :NF])
    return full


# revision 37
# speedup vs baseline: 1.0343x; 1.0343x over previous
"""Trainium2 Bass kernel for nn_ConvFCLIFNet.

Pipeline: x_seq (T=64, B=512, 1, 28, 28) -> conv2x2(valid) -> FC(729) -> LIF
scan over T -> spike sequence (T, B, 729) in {0.0, 1.0}.

Strategy (v4)
-------------
- The 2x2 conv is applied HOST-side (f32, exact): the device matmul then
  contracts over 729 conv pixels + 1 bias row = 730 rows -> 6 k-tiles of
  (5x128 + 90) instead of 785 -> 7. That cuts tensor-engine time and x
  traffic by ~14% each.
- fp16 x and weights: the PE truncates f32 ("f32r") operands to e10m11
  internally anyway; fp16 is the same 11-bit mantissa (RNE, unbiased vs
  f32r's truncation) at HALF the HBM bytes. Verified by numpy simulation:
  ~1.7k spike flips vs a ~2.9k budget at the 2e-2 rel-err gate.
- y*0.5 = x_aug @ W_aug with the 1/tau fold and bias row (row 729) baked
  into W_aug host-side, as in v2.
- Data-parallel over 8 NeuronCores: 64 samples each.
- T=64 in quanta sized so the PE is fed exactly as fast as the startup
  DMA pipe (~340 GB/s) can deliver x, and the tail tapers to a 2-step
  final quantum (short serial LIF+DMA epilogue). Within a quantum the
  matmul loop is 512-col-chunk-outer (8 timesteps), j middle, k-tile
  inner into single-bank PSUM tiles; Scalar drains each (chunk, j) to
  an SBUF y-buffer immediately, so the LIF scan can start on a chunk's
  timesteps while the PE is still on the next chunk.
- LIF scan: ONE custom DVE op per timestep (sentinel-encoded state q,
  q = (w >= 1) ? SENT : 0.5*w) over a 32-slot state ring. The raw f32
  ring is DMA'd out in small batches ISSUED ON THE SYNC QUEUE with a
  2-quantum emission lag early (LIF semaphores pre-cleared -> no
  head-of-line blocking of x prefetches) collapsing to a 1-quantum lag
  in the tail (with x prefetched 2 quanta ahead there), so the final
  timesteps' output pipelines behind the LIF chain instead of draining
  serially after it. GpSimd stays fully idle, which removes its
  expensive DGE teardown drain. Host decodes spike = (q == SENT).
- ~34 warm-up matmuls on a zeroed scratch tile run while the first x
  quantum loads, so the PE's HAM clock-gate reaches 2.4 GHz before real
  work starts.
"""
import numpy as np

import concourse.bacc as bacc
import concourse.mybir as mybir
import concourse.tile as tile
from concourse.bass_utils import run_bass_kernel_spmd

# ---------------------------------------------------------------- constants
T, B, H, W = 64, 512, 28, 28
NF = 729                # fc features (= conv pixels 27*27)
NROWS = NF + 1          # 730 contraction rows (conv pixels + bias)
NCORES = 8
BS = B // NCORES        # 64 samples per core
NJ = 6                  # feature chunks of 128 (768 padded)
KT = 6                  # contraction k-tiles: 5 x 128 + 90
KROW = 128              # rows per k-tile (kt < 5)
KTAIL = NROWS - 5 * KROW   # 90 (89 pixels + bias row)
QUANTA = (8, 8, 12, 16, 12, 4, 2, 2)  # timesteps per quantum: small leading
                                   # quanta keep the PE fed while the DMA
                                   # pipe ramps; small tail quanta shrink
                                   # the serial LIF+DMA epilogue
OUTB = 4                # timesteps per output DMA
# which quanta's output DMAs are emitted during which loop iteration
# (lag-2 early, lag-1 for the tail; the final quantum is emitted post-loop)
EMIT_AT = {2: (0,), 3: (1,), 4: (2,), 5: (3, 4), 6: (5,), 7: (6,)}
NWARM = 34              # PE warm-up matmuls
SENT = float(2 ** 20)

_CACHE = {}


def _chunks(ns, step=512):
    """(offset, width) pairs covering ns columns in <=step slices."""
    out = []
    o = 0
    while o < ns:
        out.append((o, min(step, ns - o)))
        o += step
    return out

# ------------------------------------------------------------ custom DVE op

def _register_lif_op():
    from concourse.dve_spec import Spec, Src0, Src1, C0, C1, Zero, One, select, eq, lower
    from concourse.dve_uop import DveOpSpec
    from concourse import dve_ops

    name = "LIF_STEP_ANT"
    for op in dve_ops.OPS:
        if op.name == name:
            return op

    def _ref(in0, in1, s0, s1, imm2=None):
        u = np.where(in1 == s0, 0.0, in1).astype(np.float32)
        w = (in0 + u).astype(np.float32)
        return np.where(w >= 1.0, np.float32(s0), (w * np.float32(s1)).astype(np.float32))

    _u = select(eq(Src1, C0), Zero, Src1)
    _w = Src0 + _u
    spec = Spec(body=select(_w >= One, C0, _w * C1), reference=_ref)

    row = dve_ops._CUSTOM_DVE_ROW_BASE + len(dve_ops.OPS)
    assert row < 0x20
    dve_ops._SUB_OPCODE_FOR_NAME[name] = row
    shas = {}
    for ver in ("v3", "v4"):
        s = DveOpSpec(name=name, opcode=row, uops=lower(spec, ver=ver), rd1_en=True)
        shas[ver] = s.sha(ver)
    op = dve_ops.DveOp(name, spec, subdim=False, uops_sha=shas)
    dve_ops.OPS.append(op)
    dve_ops.CUSTOM_DVE_SPECS[name] = spec
    return op

# ------------------------------------------------------------- device build

def _build():
    lif = _register_lif_op()
    nc = bacc.Bacc(None, target_bir_lowering=False, debug=False)
    f32, f16, f8 = mybir.dt.float32, mybir.dt.float16, mybir.dt.float8e5
    NTOT = T * BS  # 4096 moving columns total
    with tile.TileContext(nc) as tc:
        with tc.tile_pool(name="dram", bufs=1, space="DRAM") as dram, \
             tc.tile_pool(name="consts", bufs=1) as consts, \
             tc.tile_pool(name="xpool", bufs=3) as xpool, \
             tc.tile_pool(name="ypool", bufs=3) as ypool, \
             tc.tile_pool(name="pspool", bufs=8, space="PSUM") as pspool:
            # x pre-tiled host-side to [partition, kt, col] so a quantum load
            # is ONE 3D access pattern; full-128-partition DMAs only
            x_in = dram.tile([128, KT, NTOT], f16, kind="ExternalInput",
                             name="x_in", uniquify=False)
            w_in = dram.tile([NJ, 128, KT, 128], f16, kind="ExternalInput",
                             name="w_in", uniquify=False)
            out = dram.tile([128, T, NJ, BS], f32, kind="ExternalOutput",
                            name="out", uniquify=False)

            wsb = consts.tile([128, NJ, KT, 128], f16)

            # 32-slot ring of LIF state; slot t%32 holds q after step t.
            # Slot 31 doubles as the zero initial state. The raw sentinel-
            # encoded f32 state is DMA'd out directly (no on-device spike
            # decode); the host decodes spike = (q == SENT). 32 slots (2
            # quanta of WAR slack) let the output DMAs be EMITTED two
            # quanta late on the Sync queue: their LIF semaphores are
            # already cleared when they reach the queue head, so they
            # never head-of-line-block the x prefetches, and GpSimd stays
            # fully idle (no expensive DGE teardown drain).
            NRING = 32
            qring = consts.tile([128, NRING, NJ * BS], f32)
            nc.vector.memset(qring[:, NRING - 1, :], 0.0)

            def emit_outs(c, ob=OUTB):
                """Output DMAs for quantum c's timesteps, ob per descriptor."""
                for tb in range(t0s[c], t0s[c + 1], ob):
                    nc.sync.dma_start(
                        out=out[:, tb:tb + ob, :, :],
                        in_=qring[:, tb % NRING:tb % NRING + ob, :])

            # PE warm-up scratch (zeros; results discarded)
            warm = consts.tile([128, 128], f16)
            nc.vector.memset(warm[:, :], 0.0)

            t0s = np.cumsum((0,) + QUANTA)

            def load_x(c, kt_split=False):
                """x DMA for quantum c (optionally split by k-tile so the
                first matmuls can start on partial data)."""
                tq = QUANTA[c]
                c0 = t0s[c] * BS
                x_sb = xpool.tile([128, KT, 16 * BS], f16, name="x_sb", tag="x")
                splits = ((0, 3), (3, KT)) if kt_split else ((0, KT),)
                for k0, k1 in splits:
                    nc.sync.dma_start(
                        out=x_sb[:, k0:k1, 0:tq * BS],
                        in_=x_in[:, k0:k1, c0:c0 + tq * BS],
                    )
                return x_sb

            # startup order: w0 first (smallest blocker for the j=0 chain),
            # then x0 split by k-tile, then the remaining weights; warm-up
            # matmuls keep the PE HAM busy while those DMAs land.
            wps = pspool.tile([128, 512], f32, name="ps", tag="ps")
            for i in range(NWARM):
                nc.tensor.matmul(wps[:, 0:128], lhsT=warm[0:128, 0:128],
                                 rhs=warm[:, 0:128], start=True, stop=True)
            nc.sync.dma_start(out=wsb[:, 0, :, :], in_=w_in[0])
            x0 = load_x(0, kt_split=True)
            for j in range(1, NJ):
                nc.sync.dma_start(out=wsb[:, j, :, :], in_=w_in[j])
            x_tiles = {0: x0}

            for c, tq in enumerate(QUANTA):
                ns = tq * BS                      # moving cols this quantum
                x_sb = x_tiles.pop(c)
                # prefetch one quantum ahead early on, two ahead in the
                # tail so the small late x loads are never queued behind
                # lag-1 output emissions on the Sync queue
                for ahead in (1, 2):
                    cn = c + ahead
                    if (cn < len(QUANTA) and cn not in x_tiles
                            and (ahead == 1 or c >= 3)):
                        x_tiles[cn] = load_x(cn)
                # Output emission schedule: lag-2 for the big early quanta
                # (their long LIF chains need the slack so the DMAs never
                # wait at the Sync queue head), collapsing to lag-1 for the
                # small tail quanta (their LIF semaphores clear quickly) so
                # the 3.9 MB of tail output PIPELINES behind the LIF chain
                # instead of draining serially after it. Small ob batches in
                # the tail stream each pair of steps as soon as LIF lands.
                for cq in EMIT_AT.get(c, ()):
                    emit_outs(cq, ob=4 if QUANTA[cq] > 4 else 2)
                y_sb = ypool.tile([128, NJ, 16 * BS], f32, name="y_sb", tag="y")
                # chunk-outer: each 512-col chunk (8 timesteps) finishes all
                # 6 j accumulations + drains before the next chunk, so the
                # LIF scan starts half a quantum earlier
                for o, wd in _chunks(ns):
                    for j in range(NJ):
                        ps = pspool.tile([128, 512], f32, name="ps", tag="ps")
                        for kt in range(KT):
                            rows = KROW if kt < 5 else KTAIL
                            nc.tensor.matmul(
                                ps[:, 0:wd],
                                lhsT=wsb[0:rows, j, kt, :],
                                rhs=x_sb[0:rows, kt, o:o + wd],
                                start=(kt == 0), stop=(kt == KT - 1),
                            )
                        # drain PSUM slot to SBUF right away (GpSimd can't
                        # read PSUM on TRN2, so Scalar does all drains)
                        nc.scalar.copy(out=y_sb[:, j, o:o + wd],
                                       in_=ps[:, 0:wd])

                # LIF scan over this quantum's timesteps (from SBUF)
                for tl in range(tq):
                    t = t0s[c] + tl
                    nc.vector._custom_dve(
                        lif,
                        out=qring[:, t % NRING, :],
                        in0=y_sb[:, :, tl * BS:(tl + 1) * BS],
                        in1=qring[:, (t - 1) % NRING, :],
                        s0=SENT, s1=0.5,
                    )

            # tail: the final quantum's outputs, small batches so the
            # last LIF -> DMA chain is short
            emit_outs(len(QUANTA) - 1, ob=2)
    nc.compile()
    return nc

# --------------------------------------------------------------- host side

def _prep_weights(fc_w, fc_b):
    """w_in [NJ, 128, KT, 128] fp16: contraction rows (730 = 5*128 + 90) in
    partition-major per-j blocks; cols = 768 features (729 + pad); scaled by
    0.5 (tau fold). Bias lives at global row 729 (kt=5, p=89)."""
    w_aug = np.zeros((KT * 128, NJ * 128), np.float32)
    w_aug[:NF, :NF] = 0.5 * fc_w.astype(np.float32).T   # [in_pixel, out_feat]
    w_aug[NF, :NF] = 0.5 * fc_b.astype(np.float32)
    w4 = w_aug.reshape(KT, 128, NJ, 128).astype(np.float16)
    return np.ascontiguousarray(w4.transpose(2, 1, 0, 3))  # [NJ, 128, KT, 128]

def _prep_x(x_seq, conv_w):
    """Host conv (f32, exact) + per-core pre-tiled inputs
    [NCORES][128, KT, T*BS] fp16, cols t-major.

    Partition p of k-tile kt holds contraction row kt*128 + p (rows 0..729 =
    729 conv pixels + bias row of ones); unused pad partitions are zero."""
    cw = conv_w.reshape(2, 2).astype(np.float32)
    x = np.asarray(x_seq, dtype=np.float32).reshape(T, B, H, W)
    xc = (cw[0, 0] * x[:, :, :27, :27] + cw[0, 1] * x[:, :, :27, 1:] +
          cw[1, 0] * x[:, :, 1:, :27] + cw[1, 1] * x[:, :, 1:, 1:])
    xc = xc.reshape(T, NCORES, BS, NF).astype(np.float16)
    xt = xc.transpose(1, 3, 0, 2).reshape(NCORES, NF, T * BS)
    xp = np.zeros((NCORES, 128, KT, T * BS), np.float16)
    for kt in range(KT):
        g0 = kt * KROW
        npx = min(KROW, NF - g0)                      # pixel rows in this kt
        xp[:, 0:npx, kt, :] = xt[:, g0:g0 + npx, :]
    xp[:, NF - 5 * KROW, 5, :] = 1.0                  # bias row (global 729)
    return xp

def kernel(x_seq, conv_w, fc_w, fc_b):
    # coerce to numpy up front (the caller may pass jax arrays; all host
    # prep must stay on the CPU)
    x_seq = np.asarray(x_seq, dtype=np.float32)
    conv_w = np.asarray(conv_w, dtype=np.float32)
    fc_w = np.asarray(fc_w, dtype=np.float32)
    fc_b = np.asarray(fc_b, dtype=np.float32)
    if "nc" not in _CACHE:
        _CACHE["nc"] = _build()
    nc = _CACHE["nc"]
    w_in = _prep_weights(fc_w, fc_b)
    xp = _prep_x(x_seq, conv_w)
    in_maps = [{"x_in": np.ascontiguousarray(xp[c]), "w_in": w_in}
               for c in range(NCORES)]
    res = run_bass_kernel_spmd(nc, in_maps, core_ids=list(range(NCORES)))
    _CACHE["last_res"] = res
    full = np.empty((T, B, NF), np.float32)
    for c in range(NCORES):
        o = res.results[c]["out"]                     # [128, T, NJ, BS] f32
        # spike decode: q == SENT exactly iff the neuron fired this step
        s = (o == np.float32(SENT)).astype(np.float32)
        # feature f = j*128 + p ; sample b
        full[:, c * BS:(c + 1) * BS, :] = (
            s.transpose(1, 3, 2, 0).reshape(T, BS, NJ * 128)[:, :, :NF])
    return full
